# revision 9
# baseline (speedup 1.0000x reference)
"""Trainium2 Bass kernel for the pose-estimation loss (pm / t_center / t_depth).

Strategy
--------
pm[n] = mean_p | (pred_R[n]-gt_R[n]) @ obj_points[obj_id[n], p] |_1 / diam[obj_id[n]]

The data-dependent gather obj_points[obj_id] is folded into the matmul:
    Y[(i,n), p] = sum_{o,j} A[(o,j),(i,n)] * B[(o,j), p]
with A[(o,j),(i,n)] = [obj_id[n]==o] * dR[n,i,j]   (24 x 384, built on host)
     B[(o,j), p]    = obj_points[o, p, j]          (24 x 12500 per core)

The bottleneck is draining PSUM through abs+sum.  Only DVE and ACT can read
PSUM, each at 1 elem/lane/cycle (a DVE instruction may read at most ONE
non-scalar input from PSUM — verifier NCC_IBVF027 — so no 2-stream tricks).
Peak drain = DVE tensor_reduce(abs) @0.96GHz + ACT activation(Abs, accum_out)
@1.2GHz, both fully fused into the accumulation and running concurrently on
disjoint PSUM banks:

  - DVE tiles [128,2,512] (banks 0-3, double-buffered): tensor_reduce with
    apply_absolute_value, writing one acc column per instruction.
  - ACT tiles [128,2,512] (banks 4-7, double-buffered): activation(Abs)
    with accum_out, one acc column per instruction.

PE quadrants: q0/q1 feed the DVE tiles, q2/q3 feed the ACT tiles (4 matmuls
concurrent in distinct PE row-groups; K=24 so a row-group fits 32 rows).
Point split between the engines matches their drain rates.

Per core output: out[128, 3] = [pm partial sum, t_center, t_depth].
Host: pm = sum_over_cores(out[:,0]) / 100000 / diam[obj_id].
"""

import os
import sys

import numpy as np

os.environ.setdefault("MYCRO_LOCAL_CACHE", "1")
if "/opt/trn_rl_repo" not in sys.path:
    sys.path.insert(0, "/opt/trn_rl_repo")

# ---- problem constants (hardcoded, must match the reference) ----
N_SAMPLES = 128
NUM_OBJECTS = 8
NUM_POINTS = 100000
N_CORES = 8

PTS_PER_CORE = NUM_POINTS // N_CORES  # 12500
ICHUNKS = 3                           # (i) coordinate chunks of 128 samples

# DVE quadrants (q0, q1): 2816 columns each; chunks of 512 (+ one 256).
DVE_CHUNKS = [512] * 5 + [256]
DVE_COLS = sum(DVE_CHUNKS)            # 2816
# ACT quadrants (q2, q3): 3434 columns each; chunks of 512 (+ one 362).
ACT_CHUNKS = [512] * 6 + [362]
ACT_COLS = sum(ACT_CHUNKS)            # 3434
assert 2 * DVE_COLS + 2 * ACT_COLS == PTS_PER_CORE

N_ACC = ICHUNKS * (len(DVE_CHUNKS) + len(ACT_CHUNKS))  # 39 accum columns

_CACHE = {}


def _build_module():
    """Build + compile the single-core Bass program (same program on all cores)."""
    if "nc" in _CACHE:
        return _CACHE["nc"]

    from contextlib import ExitStack

    import concourse.bass as bass  # noqa: F401  (import registers engines)
    import concourse.tile as tile
    from concourse import bacc, mybir

    f32 = mybir.dt.float32
    bf16 = mybir.dt.bfloat16

    nc = bacc.Bacc("TRN2", target_bir_lowering=False, debug=False)

    A_COLS = ICHUNKS * 128            # 384
    amat = nc.dram_tensor("amat", [24, A_COLS], bf16, kind="ExternalInput").ap()
    bq0 = nc.dram_tensor("bq0", [24, DVE_COLS], bf16, kind="ExternalInput").ap()
    bq1 = nc.dram_tensor("bq1", [24, DVE_COLS], bf16, kind="ExternalInput").ap()
    bq2 = nc.dram_tensor("bq2", [24, ACT_COLS], bf16, kind="ExternalInput").ap()
    bq3 = nc.dram_tensor("bq3", [24, ACT_COLS], bf16, kind="ExternalInput").ap()
    tsite = nc.dram_tensor("tsite", [128, 6], f32, kind="ExternalInput").ap()
    out = nc.dram_tensor("out", [128, 3], f32, kind="ExternalOutput").ap()

    with ExitStack() as ctx:
        tc = ctx.enter_context(tile.TileContext(nc))
        const = ctx.enter_context(tc.tile_pool(name="const", bufs=1))
        # v_ps / t_ps tags each get 2 bufs of [128,2,512] = 4 banks per engine.
        psum = ctx.enter_context(tc.tile_pool(name="psum", bufs=2, space="PSUM"))

        a_sb = const.tile([128, A_COLS], bf16)
        b_sb = const.tile([128, ACT_COLS], bf16)
        ts_sb = const.tile([128, 6], f32)
        acc = const.tile([128, N_ACC], f32)
        asum = const.tile([128, 2, 512], bf16)   # ACT dummy out
        out_sb = const.tile([128, 3], f32)
        warm = const.tile([128, 1], f32)
        wmm = const.tile([128, 640], bf16)
        d_sb = const.tile([128, 3], f32)

        # Warm up the ACT table set (Abs) so the ~2.7us table load overlaps DMA.
        nc.vector.memset(warm, 0.0)
        nc.scalar.activation(out=warm, in_=warm, func=mybir.ActivationFunctionType.Abs)

        # DMA: A replicated to the 4 PE row-groups (scalar HWDGE ring); B per
        # quadrant on the sync ring, first-chunk pieces first so compute
        # starts early; tsite via gpsimd (tiny, off the HWDGE rings).
        for g in range(4):
            nc.scalar.dma_start(out=a_sb[32 * g : 32 * g + 24, :], in_=amat)
        nc.sync.dma_start(out=b_sb[0:24, 0:1024], in_=bq0[:, 0:1024])
        nc.sync.dma_start(out=b_sb[32:56, 0:1024], in_=bq1[:, 0:1024])
        nc.sync.dma_start(out=b_sb[64:88, 0:1024], in_=bq2[:, 0:1024])
        nc.sync.dma_start(out=b_sb[96:120, 0:1024], in_=bq3[:, 0:1024])
        nc.sync.dma_start(out=b_sb[0:24, 1024:DVE_COLS], in_=bq0[:, 1024:])
        nc.sync.dma_start(out=b_sb[32:56, 1024:DVE_COLS], in_=bq1[:, 1024:])
        nc.sync.dma_start(out=b_sb[64:88, 1024:ACT_COLS], in_=bq2[:, 1024:])
        nc.sync.dma_start(out=b_sb[96:120, 1024:ACT_COLS], in_=bq3[:, 1024:])
        nc.gpsimd.dma_start(out=ts_sb, in_=tsite)

        # HAM warm-up: dummy matmuls on zeros while the DMAs land, so the
        # real matmuls run at 2.4 GHz instead of the cold 1.2 GHz.
        nc.vector.memset(wmm, 0.0)
        for _ in range(8):
            wps = psum.tile([128, 2, 512], f32, tag="t_ps")
            nc.tensor.matmul(
                wps[:, 0, :], lhsT=wmm[0:24, 0:128], rhs=wmm[0:24, 128:640],
                start=True, stop=True,
            )

        # t_site losses (tiny): d = gt - pred; t_center = |d0|+|d1|; t_depth = |d2|
        nc.vector.tensor_sub(d_sb, ts_sb[:, 0:3], ts_sb[:, 3:6])
        nc.vector.tensor_reduce(
            out=out_sb[:, 1:2], in_=d_sb[:, 0:2], axis=mybir.AxisListType.X,
            op=mybir.AluOpType.add, apply_absolute_value=True,
        )
        nc.vector.tensor_reduce(
            out=out_sb[:, 2:3], in_=d_sb[:, 2:3], axis=mybir.AxisListType.X,
            op=mybir.AluOpType.add, apply_absolute_value=True,
        )

        col = 0
        for i in range(ICHUNKS):
            ai = slice(i * 128, (i + 1) * 128)
            # Interleave DVE chunks (6) and ACT chunks (7) so both engines
            # stay fed.
            order = []
            for k in range(len(ACT_CHUNKS)):
                order.append(("a", k))
                if k < len(DVE_CHUNKS):
                    order.append(("v", k))
            for kind, k in order:
                off = 512 * k
                if kind == "v":
                    w = DVE_CHUNKS[k]
                    v_ps = psum.tile([128, 2, 512], f32)
                    nc.tensor.matmul(
                        v_ps[:, 0, 0:w], lhsT=a_sb[0:24, ai],
                        rhs=b_sb[0:24, off : off + w],
                        start=True, stop=True, tile_position=(0, 0),
                    )
                    nc.tensor.matmul(
                        v_ps[:, 1, 0:w], lhsT=a_sb[32:56, ai],
                        rhs=b_sb[32:56, off : off + w],
                        start=True, stop=True, tile_position=(32, 0),
                    )
                    nc.vector.tensor_reduce(
                        out=acc[:, col : col + 1], in_=v_ps[:, :, 0:w],
                        axis=mybir.AxisListType.XY, op=mybir.AluOpType.add,
                        apply_absolute_value=True,
                    )
                else:
                    w = ACT_CHUNKS[k]
                    t_ps = psum.tile([128, 2, 512], f32)
                    nc.tensor.matmul(
                        t_ps[:, 0, 0:w], lhsT=a_sb[64:88, ai],
                        rhs=b_sb[64:88, off : off + w],
                        start=True, stop=True, tile_position=(64, 0),
                    )
                    nc.tensor.matmul(
                        t_ps[:, 1, 0:w], lhsT=a_sb[96:120, ai],
                        rhs=b_sb[96:120, off : off + w],
                        start=True, stop=True, tile_position=(96, 0),
                    )
                    nc.scalar.activation(
                        out=asum[:, :, 0:w], in_=t_ps[:, :, 0:w],
                        func=mybir.ActivationFunctionType.Abs,
                        accum_out=acc[:, col : col + 1],
                    )
                col += 1

        nc.vector.tensor_reduce(
            out=out_sb[:, 0:1], in_=acc[:, 0:col], axis=mybir.AxisListType.X,
            op=mybir.AluOpType.add,
        )
        nc.sync.dma_start(out=out, in_=out_sb)

    nc.compile()
    _CACHE["nc"] = nc
    return nc


def _prepare_in_maps(obj_id, gt_cam_R_m2c, pred_cam_R_m2c, gt_cam_t_m2c_site,
                     pred_cam_t_m2c_site, obj_points, obj_diameters):
    obj_id = np.asarray(obj_id).astype(np.int64)
    dR = (np.asarray(pred_cam_R_m2c, np.float32)
          - np.asarray(gt_cam_R_m2c, np.float32))          # [N, 3, 3] (i, j)
    pts = np.asarray(obj_points, np.float32)               # [8, P, 3]

    import ml_dtypes

    # A[(o,j), (i,n)] = [obj_id[n]==o] * dR[n, i, j]
    afull = np.zeros((NUM_OBJECTS, 3, 3, N_SAMPLES), np.float32)  # [o, j, i, n]
    afull[obj_id, :, :, np.arange(N_SAMPLES)] = dR.transpose(0, 2, 1)  # [n, j, i]
    a24 = afull.reshape(NUM_OBJECTS * 3, 3 * N_SAMPLES)    # rows (o,j), cols i*128+n
    a_host = np.ascontiguousarray(a24).astype(ml_dtypes.bfloat16)

    # B rows (o,j), cols p
    b24 = pts.transpose(0, 2, 1).reshape(NUM_OBJECTS * 3, NUM_POINTS)

    ts_host = np.concatenate(
        [np.asarray(gt_cam_t_m2c_site, np.float32),
         np.asarray(pred_cam_t_m2c_site, np.float32)], axis=1)  # [128, 6]

    in_maps = []
    for c in range(N_CORES):
        cols = b24[:, c * PTS_PER_CORE : (c + 1) * PTS_PER_CORE]
        o0 = DVE_COLS
        o1 = 2 * DVE_COLS
        o2 = 2 * DVE_COLS + ACT_COLS
        in_maps.append({
            "amat": a_host,
            "bq0": np.ascontiguousarray(cols[:, 0:o0]).astype(ml_dtypes.bfloat16),
            "bq1": np.ascontiguousarray(cols[:, o0:o1]).astype(ml_dtypes.bfloat16),
            "bq2": np.ascontiguousarray(cols[:, o1:o2]).astype(ml_dtypes.bfloat16),
            "bq3": np.ascontiguousarray(cols[:, o2:]).astype(ml_dtypes.bfloat16),
            "tsite": ts_host,
        })
    return in_maps, obj_id, np.asarray(obj_diameters, np.float32)


def _postprocess(results, obj_id, obj_diameters):
    pm_sum = np.zeros(N_SAMPLES, np.float64)
    for c in range(N_CORES):
        pm_sum += results[c]["out"][:, 0].astype(np.float64)
    pm = (pm_sum / NUM_POINTS / obj_diameters[obj_id].astype(np.float64)).astype(
        np.float32)
    t_center = results[0]["out"][:, 1].astype(np.float32)
    t_depth = results[0]["out"][:, 2].astype(np.float32)
    return pm, t_center, t_depth


def run(inputs, trace=False):
    """Run on the 8 NeuronCores. Returns ((pm, t_center, t_depth), BassKernelResults)."""
    from concourse.bass_utils import run_bass_kernel_spmd

    nc = _build_module()
    in_maps, obj_id, diam = _prepare_in_maps(**inputs)
    res = run_bass_kernel_spmd(nc, in_maps, list(range(N_CORES)), trace=trace)
    return _postprocess(res.results, obj_id, diam), res


def run_sim(inputs):
    """CoreSim path (numerics check without hardware)."""
    from concourse.bass_interp import CoreSim

    nc = _build_module()
    in_maps, obj_id, diam = _prepare_in_maps(**inputs)
    results = []
    for c in range(N_CORES):
        sim = CoreSim(nc)
        for name, val in in_maps[c].items():
            sim.tensor(name)[:] = val
        sim.simulate(check_with_hw=False)
        results.append({"out": np.array(sim.tensor("out"))})
    return _postprocess(results, obj_id, diam)


def kernel(**inputs):
    (pm, t_center, t_depth), _ = run(inputs, trace=False)
    return pm, t_center, t_depth


# revision 10
# speedup vs baseline: 1.1456x; 1.1456x over previous
"""Trainium2 Bass kernel for the pose-estimation loss (pm / t_center / t_depth).

Strategy
--------
pm[n] = mean_p | (pred_R[n]-gt_R[n]) @ obj_points[obj_id[n], p] |_1 / diam[obj_id[n]]

The data-dependent gather obj_points[obj_id] is folded into the matmul:
    Y[(i,n), p] = sum_{o,j} A[(o,j),(i,n)] * B[(o,j), p]
with A[(o,j),(i,n)] = [obj_id[n]==o] * dR[n,i,j]   (24 x 384, built on host)
     B[(o,j), p]    = obj_points[o, p, j]          (24 x 12500 per core)

The bottleneck is draining PSUM through abs+sum.  Only DVE and ACT can read
PSUM, each at 1 elem/lane/cycle (a DVE instruction may read at most ONE
non-scalar input from PSUM — verifier NCC_IBVF027 — so no 2-stream tricks).
Measured drain rates on HW: DVE tensor_reduce(abs) ~111 G elem/s, ACT
activation(Abs, accum_out) ~99 G elem/s (the ACTIVATION_READ_ACCUMULATOR
mostly overlaps the activate tail).  Both run concurrently on disjoint PSUM
banks, fully fused into per-instruction accumulator columns:

  - DVE tiles [128,2,512] f32 (2 banks, double-buffered = 4 banks):
    tensor_reduce(apply_absolute_value, XY) -> acc column.
  - ACT tiles [128,2,512] f32 (2 banks, double-buffered = 4 banks):
    activation(Abs, accum_out) -> acc column.

PE quadrants: q0/q1 feed DVE tiles, q2/q3 feed ACT tiles; 2-4 matmuls run
concurrently in distinct PE row-groups (K=24 fits a 32-row group).  The PE
runs at the cold 1.2 GHz clock on this part (HAM never engages), so warm-up
matmuls are pure delay and are omitted; even cold, the PE outruns the
drains at ~3-way row-group overlap.

Host packs A (replicated to the 4 row-groups) and the per-quadrant B blocks
into one [128, 384+3312] bf16 DRAM tensor; the first DMA piece carries A
plus the first 512 B columns of every quadrant so compute starts ~1.5us in.

Per core output: out[128, 3] = [pm partial sum, t_center, t_depth].
Host: pm = sum_over_cores(out[:,0]) / 100000 / diam[obj_id].
"""

import os
import sys

import numpy as np

os.environ.setdefault("MYCRO_LOCAL_CACHE", "1")
if "/opt/trn_rl_repo" not in sys.path:
    sys.path.insert(0, "/opt/trn_rl_repo")

# ---- problem constants (hardcoded, must match the reference) ----
N_SAMPLES = 128
NUM_OBJECTS = 8
NUM_POINTS = 100000
N_CORES = 8

PTS_PER_CORE = NUM_POINTS // N_CORES  # 12500
ICHUNKS = 3                           # (i) coordinate chunks of 128 samples
A_COLS = ICHUNKS * 128                # 384

# DVE quadrants (q0, q1) and ACT quadrants (q2, q3): column counts matched
# to the measured drain rates (DVE ~111 G elem/s vs ACT ~99 G elem/s).
DVE_CHUNKS = [512] * 6 + [240]
DVE_COLS = sum(DVE_CHUNKS)            # 3312
ACT_CHUNKS = [512] * 5 + [378]
ACT_COLS = sum(ACT_CHUNKS)            # 2938
assert 2 * DVE_COLS + 2 * ACT_COLS == PTS_PER_CORE

AB_COLS = A_COLS + DVE_COLS           # 3696 (q2/q3 rows zero-padded at the end)
N_ACC = ICHUNKS * (len(DVE_CHUNKS) + len(ACT_CHUNKS))  # 39 accum columns

_CACHE = {}


def _build_module():
    """Build + compile the single-core Bass program (same program on all cores)."""
    if "nc" in _CACHE:
        return _CACHE["nc"]

    from contextlib import ExitStack

    import concourse.bass as bass  # noqa: F401  (import registers engines)
    import concourse.tile as tile
    from concourse import bacc, mybir

    f32 = mybir.dt.float32
    bf16 = mybir.dt.bfloat16

    nc = bacc.Bacc("TRN2", target_bir_lowering=False, debug=False)

    abmat = nc.dram_tensor("abmat", [128, AB_COLS], bf16, kind="ExternalInput").ap()
    tsite = nc.dram_tensor("tsite", [128, 6], f32, kind="ExternalInput").ap()
    out = nc.dram_tensor("out", [128, 3], f32, kind="ExternalOutput").ap()

    with ExitStack() as ctx:
        tc = ctx.enter_context(tile.TileContext(nc))
        const = ctx.enter_context(tc.tile_pool(name="const", bufs=1))
        # v_ps / t_ps tags each get 2 bufs of [128,2,512] = 4 banks per engine.
        psum = ctx.enter_context(tc.tile_pool(name="psum", bufs=2, space="PSUM"))

        ab_sb = const.tile([128, AB_COLS], bf16)
        a_sb = ab_sb[:, 0:A_COLS]
        b_sb = ab_sb[:, A_COLS:]
        ts_sb = const.tile([128, 6], f32)
        acc = const.tile([128, N_ACC], f32)
        asum = const.tile([128, 2, 512], bf16)   # ACT dummy out
        out_sb = const.tile([128, 3], f32)
        warm = const.tile([128, 1], f32)
        d_sb = const.tile([128, 3], f32)

        # Warm up the ACT table set (Abs) so the ~1.3us table load overlaps DMA.
        nc.vector.memset(warm, 0.0)
        nc.scalar.activation(out=warm, in_=warm, func=mybir.ActivationFunctionType.Abs)

        # DMA: piece 1 carries A plus the first 512 B columns of every
        # quadrant (the column space is shared across the 4 row-groups), so
        # the first chunk of both engine paths can start immediately.  The
        # remaining pieces alternate between the two HWDGE rings; tsite goes
        # via gpsimd (tiny, off the HWDGE rings).
        nc.sync.dma_start(out=ab_sb[:, 0 : A_COLS + 512],
                          in_=abmat[:, 0 : A_COLS + 512])
        nc.scalar.dma_start(out=ab_sb[:, A_COLS + 512 : A_COLS + 1536],
                            in_=abmat[:, A_COLS + 512 : A_COLS + 1536])
        nc.sync.dma_start(out=ab_sb[:, A_COLS + 1536 : A_COLS + 2560],
                          in_=abmat[:, A_COLS + 1536 : A_COLS + 2560])
        nc.scalar.dma_start(out=ab_sb[:, A_COLS + 2560 :],
                            in_=abmat[:, A_COLS + 2560 :])
        nc.gpsimd.dma_start(out=ts_sb, in_=tsite)

        col = 0
        for i in range(ICHUNKS):
            ai = slice(i * 128, (i + 1) * 128)
            # Interleave DVE chunks (7) and ACT chunks (6) so both engines
            # stay fed.
            order = []
            for k in range(len(DVE_CHUNKS)):
                order.append(("v", k))
                if k < len(ACT_CHUNKS):
                    order.append(("a", k))
            for kind, k in order:
                off = 512 * k
                if kind == "v":
                    w = DVE_CHUNKS[k]
                    v_ps = psum.tile([128, 2, 512], f32)
                    nc.tensor.matmul(
                        v_ps[:, 0, 0:w], lhsT=a_sb[0:24, ai],
                        rhs=b_sb[0:24, off : off + w],
                        start=True, stop=True, tile_position=(0, 0),
                    )
                    nc.tensor.matmul(
                        v_ps[:, 1, 0:w], lhsT=a_sb[32:56, ai],
                        rhs=b_sb[32:56, off : off + w],
                        start=True, stop=True, tile_position=(32, 0),
                    )
                    nc.vector.tensor_reduce(
                        out=acc[:, col : col + 1], in_=v_ps[:, :, 0:w],
                        axis=mybir.AxisListType.XY, op=mybir.AluOpType.add,
                        apply_absolute_value=True,
                    )
                else:
                    w = ACT_CHUNKS[k]
                    t_ps = psum.tile([128, 2, 512], f32)
                    nc.tensor.matmul(
                        t_ps[:, 0, 0:w], lhsT=a_sb[64:88, ai],
                        rhs=b_sb[64:88, off : off + w],
                        start=True, stop=True, tile_position=(64, 0),
                    )
                    nc.tensor.matmul(
                        t_ps[:, 1, 0:w], lhsT=a_sb[96:120, ai],
                        rhs=b_sb[96:120, off : off + w],
                        start=True, stop=True, tile_position=(96, 0),
                    )
                    nc.scalar.activation(
                        out=asum[:, :, 0:w], in_=t_ps[:, :, 0:w],
                        func=mybir.ActivationFunctionType.Abs,
                        accum_out=acc[:, col : col + 1],
                    )
                col += 1

        # t_site losses (tiny, issued after the loop so they don't block the
        # DVE FIFO on the tsite DMA): d = gt - pred; t_center = |d0|+|d1|;
        # t_depth = |d2|
        nc.vector.tensor_sub(d_sb, ts_sb[:, 0:3], ts_sb[:, 3:6])
        nc.vector.tensor_reduce(
            out=out_sb[:, 1:2], in_=d_sb[:, 0:2], axis=mybir.AxisListType.X,
            op=mybir.AluOpType.add, apply_absolute_value=True,
        )
        nc.vector.tensor_reduce(
            out=out_sb[:, 2:3], in_=d_sb[:, 2:3], axis=mybir.AxisListType.X,
            op=mybir.AluOpType.add, apply_absolute_value=True,
        )
        nc.vector.tensor_reduce(
            out=out_sb[:, 0:1], in_=acc[:, 0:col], axis=mybir.AxisListType.X,
            op=mybir.AluOpType.add,
        )
        nc.sync.dma_start(out=out, in_=out_sb)

    nc.compile()
    _CACHE["nc"] = nc
    return nc


def _prepare_in_maps(obj_id, gt_cam_R_m2c, pred_cam_R_m2c, gt_cam_t_m2c_site,
                     pred_cam_t_m2c_site, obj_points, obj_diameters):
    obj_id = np.asarray(obj_id).astype(np.int64)
    dR = (np.asarray(pred_cam_R_m2c, np.float32)
          - np.asarray(gt_cam_R_m2c, np.float32))          # [N, 3, 3] (i, j)
    pts = np.asarray(obj_points, np.float32)               # [8, P, 3]

    import ml_dtypes

    # A[(o,j), (i,n)] = [obj_id[n]==o] * dR[n, i, j]
    afull = np.zeros((NUM_OBJECTS, 3, 3, N_SAMPLES), np.float32)  # [o, j, i, n]
    afull[obj_id, :, :, np.arange(N_SAMPLES)] = dR.transpose(0, 2, 1)  # [n, j, i]
    a24 = afull.reshape(NUM_OBJECTS * 3, 3 * N_SAMPLES)    # rows (o,j), cols i*128+n

    # B rows (o,j), cols p
    b24 = pts.transpose(0, 2, 1).reshape(NUM_OBJECTS * 3, NUM_POINTS)

    ts_host = np.concatenate(
        [np.asarray(gt_cam_t_m2c_site, np.float32),
         np.asarray(pred_cam_t_m2c_site, np.float32)], axis=1)  # [128, 6]

    quad_cols = [DVE_COLS, DVE_COLS, ACT_COLS, ACT_COLS]
    in_maps = []
    for c in range(N_CORES):
        cols = b24[:, c * PTS_PER_CORE : (c + 1) * PTS_PER_CORE]
        ab = np.zeros((128, AB_COLS), np.float32)
        off = 0
        for g in range(4):
            w = quad_cols[g]
            ab[32 * g : 32 * g + 24, 0:A_COLS] = a24
            ab[32 * g : 32 * g + 24, A_COLS : A_COLS + w] = cols[:, off : off + w]
            off += w
        in_maps.append({
            "abmat": np.ascontiguousarray(ab).astype(ml_dtypes.bfloat16),
            "tsite": ts_host,
        })
    return in_maps, obj_id, np.asarray(obj_diameters, np.float32)


def _postprocess(results, obj_id, obj_diameters):
    pm_sum = np.zeros(N_SAMPLES, np.float64)
    for c in range(N_CORES):
        pm_sum += results[c]["out"][:, 0].astype(np.float64)
    pm = (pm_sum / NUM_POINTS / obj_diameters[obj_id].astype(np.float64)).astype(
        np.float32)
    t_center = results[0]["out"][:, 1].astype(np.float32)
    t_depth = results[0]["out"][:, 2].astype(np.float32)
    return pm, t_center, t_depth


def run(inputs, trace=False):
    """Run on the 8 NeuronCores. Returns ((pm, t_center, t_depth), BassKernelResults)."""
    from concourse.bass_utils import run_bass_kernel_spmd

    nc = _build_module()
    in_maps, obj_id, diam = _prepare_in_maps(**inputs)
    res = run_bass_kernel_spmd(nc, in_maps, list(range(N_CORES)), trace=trace)
    return _postprocess(res.results, obj_id, diam), res


def run_sim(inputs):
    """CoreSim path (numerics check without hardware)."""
    from concourse.bass_interp import CoreSim

    nc = _build_module()
    in_maps, obj_id, diam = _prepare_in_maps(**inputs)
    results = []
    for c in range(N_CORES):
        sim = CoreSim(nc)
        for name, val in in_maps[c].items():
            sim.tensor(name)[:] = val
        sim.simulate(check_with_hw=False)
        results.append({"out": np.array(sim.tensor("out"))})
    return _postprocess(results, obj_id, diam)


def kernel(**inputs):
    (pm, t_center, t_depth), _ = run(inputs, trace=False)
    return pm, t_center, t_depth


# revision 11
# speedup vs baseline: 1.2168x; 1.0621x over previous
"""Trainium2 Bass kernel for the pose-estimation loss (pm / t_center / t_depth).

Strategy
--------
pm[n] = mean_p | (pred_R[n]-gt_R[n]) @ obj_points[obj_id[n], p] |_1 / diam[obj_id[n]]

The data-dependent gather obj_points[obj_id] is folded into the matmul:
    Y[(i,n), p] = sum_{o,j} A[(o,j),(i,n)] * B[(o,j), p]
with A[(o,j),(i,n)] = [obj_id[n]==o] * dR[n,i,j]   (24 x 384, built on host)
     B[(o,j), p]    = obj_points[o, p, j]          (24 x 12500 per core)

The bottleneck is draining PSUM through abs+sum.  Only DVE and ACT can read
PSUM, each at 1 elem/lane/cycle (a DVE instruction may read at most ONE
non-scalar input from PSUM — verifier NCC_IBVF027 — so no 2-stream tricks).
Measured drain rates on HW: DVE tensor_reduce(abs) ~111 G elem/s, ACT
activation(Abs, accum_out) ~99 G elem/s (the ACTIVATION_READ_ACCUMULATOR
mostly overlaps the activate tail).  Both run concurrently on disjoint PSUM
banks, fully fused into per-instruction accumulator columns:

  - DVE tiles [128,2,512] f32 (2 banks, double-buffered = 4 banks):
    tensor_reduce(apply_absolute_value, XY) -> acc column.
  - ACT tiles [128,2,512] f32 (2 banks, double-buffered = 4 banks):
    activation(Abs, accum_out) -> acc column.

PE quadrants: q0/q1 feed DVE tiles, q2/q3 feed ACT tiles; 2-4 matmuls run
concurrently in distinct PE row-groups (K=24 fits a 32-row group).  The PE
runs at the cold 1.2 GHz clock on this part (HAM never engages), so warm-up
matmuls are pure delay and are omitted; even cold, the PE outruns the
drains at ~3-way row-group overlap.

Host packs A (replicated to the 4 row-groups) and the per-quadrant B blocks
into one [128, 384+3312] bf16 DRAM tensor; the first DMA piece carries A
plus the first 512 B columns of every quadrant so compute starts ~1.5us in.

Per core output: out[128, 3] = [pm partial sum, t_center, t_depth].
Host: pm = sum_over_cores(out[:,0]) / 100000 / diam[obj_id].
"""

import os
import sys

import numpy as np

os.environ.setdefault("MYCRO_LOCAL_CACHE", "1")
if "/opt/trn_rl_repo" not in sys.path:
    sys.path.insert(0, "/opt/trn_rl_repo")

# ---- problem constants (hardcoded, must match the reference) ----
N_SAMPLES = 128
NUM_OBJECTS = 8
NUM_POINTS = 100000
N_CORES = 8

PTS_PER_CORE = NUM_POINTS // N_CORES  # 12500
ICHUNKS = 3                           # (i) coordinate chunks of 128 samples
A_COLS = ICHUNKS * 128                # 384

# DVE quadrants (q0, q1) and ACT quadrants (q2, q3): column counts matched
# to the measured drain rates (DVE ~111 G elem/s vs ACT ~99 G elem/s).
DVE_CHUNKS = [512] * 6 + [240]
DVE_COLS = sum(DVE_CHUNKS)            # 3312
ACT_CHUNKS = [512] * 5 + [378]
ACT_COLS = sum(ACT_CHUNKS)            # 2938
assert 2 * DVE_COLS + 2 * ACT_COLS == PTS_PER_CORE

AB_COLS = A_COLS + DVE_COLS           # 3696 (q2/q3 rows zero-padded at the end)
N_ACC = ICHUNKS * (len(DVE_CHUNKS) + len(ACT_CHUNKS))  # 39 accum columns

_CACHE = {}


def _build_module():
    """Build + compile the single-core Bass program (same program on all cores)."""
    if "nc" in _CACHE:
        return _CACHE["nc"]

    from contextlib import ExitStack

    import concourse.bass as bass  # noqa: F401  (import registers engines)
    import concourse.tile as tile
    from concourse import bacc, mybir

    f32 = mybir.dt.float32
    bf16 = mybir.dt.bfloat16

    nc = bacc.Bacc("TRN2", target_bir_lowering=False, debug=False)

    abmat = nc.dram_tensor("abmat", [128, AB_COLS], bf16, kind="ExternalInput").ap()
    tsite = nc.dram_tensor("tsite", [128, 6], f32, kind="ExternalInput").ap()
    out = nc.dram_tensor("out", [128, 3], f32, kind="ExternalOutput").ap()

    with ExitStack() as ctx:
        tc = ctx.enter_context(tile.TileContext(nc))
        const = ctx.enter_context(tc.tile_pool(name="const", bufs=1))
        # v_ps / t_ps tags each get 2 bufs of [128,2,512] = 4 banks per engine.
        psum = ctx.enter_context(tc.tile_pool(name="psum", bufs=2, space="PSUM"))

        ab_sb = const.tile([128, AB_COLS], bf16)
        a_sb = ab_sb[:, 0:A_COLS]
        b_sb = ab_sb[:, A_COLS:]
        ts_sb = const.tile([128, 6], f32)
        acc = const.tile([128, N_ACC], f32)
        asum = const.tile([128, 2, 512], bf16)   # ACT dummy out
        out_sb = const.tile([128, 3], f32)
        warm = const.tile([128, 1], f32)
        d_sb = const.tile([128, 3], f32)

        # Warm up the ACT table set (Abs) so the ~1.3us table load overlaps DMA.
        nc.vector.memset(warm, 0.0)
        nc.scalar.activation(out=warm, in_=warm, func=mybir.ActivationFunctionType.Abs)

        # DMA: piece 1 is A alone (small, so its ~2us completion round-trip
        # gates the first LDWEIGHTS as early as possible); the first B piece
        # goes on the other HWDGE ring in parallel; remaining pieces
        # alternate rings.  No gpsimd DMA anywhere — SWDGE would add its
        # descriptor-ring init and multi-us quiesce drains to the window.
        nc.sync.dma_start(out=ab_sb[:, 0:A_COLS], in_=abmat[:, 0:A_COLS])
        nc.scalar.dma_start(out=ab_sb[:, A_COLS : A_COLS + 512],
                            in_=abmat[:, A_COLS : A_COLS + 512])
        nc.sync.dma_start(out=ab_sb[:, A_COLS + 512 : A_COLS + 1536],
                          in_=abmat[:, A_COLS + 512 : A_COLS + 1536])
        nc.scalar.dma_start(out=ab_sb[:, A_COLS + 1536 : A_COLS + 2560],
                            in_=abmat[:, A_COLS + 1536 : A_COLS + 2560])
        nc.sync.dma_start(out=ab_sb[:, A_COLS + 2560 :],
                          in_=abmat[:, A_COLS + 2560 :])
        nc.scalar.dma_start(out=ts_sb, in_=tsite)

        col = 0
        for i in range(ICHUNKS):
            ai = slice(i * 128, (i + 1) * 128)
            # Interleave DVE chunks (7) and ACT chunks (6) so both engines
            # stay fed.
            order = []
            for k in range(len(DVE_CHUNKS)):
                order.append(("v", k))
                if k < len(ACT_CHUNKS):
                    order.append(("a", k))
            for kind, k in order:
                off = 512 * k
                if kind == "v":
                    w = DVE_CHUNKS[k]
                    v_ps = psum.tile([128, 2, 512], f32)
                    nc.tensor.matmul(
                        v_ps[:, 0, 0:w], lhsT=a_sb[0:24, ai],
                        rhs=b_sb[0:24, off : off + w],
                        start=True, stop=True, tile_position=(0, 0),
                    )
                    nc.tensor.matmul(
                        v_ps[:, 1, 0:w], lhsT=a_sb[32:56, ai],
                        rhs=b_sb[32:56, off : off + w],
                        start=True, stop=True, tile_position=(32, 0),
                    )
                    nc.vector.tensor_reduce(
                        out=acc[:, col : col + 1], in_=v_ps[:, :, 0:w],
                        axis=mybir.AxisListType.XY, op=mybir.AluOpType.add,
                        apply_absolute_value=True,
                    )
                else:
                    w = ACT_CHUNKS[k]
                    t_ps = psum.tile([128, 2, 512], f32)
                    nc.tensor.matmul(
                        t_ps[:, 0, 0:w], lhsT=a_sb[64:88, ai],
                        rhs=b_sb[64:88, off : off + w],
                        start=True, stop=True, tile_position=(64, 0),
                    )
                    nc.tensor.matmul(
                        t_ps[:, 1, 0:w], lhsT=a_sb[96:120, ai],
                        rhs=b_sb[96:120, off : off + w],
                        start=True, stop=True, tile_position=(96, 0),
                    )
                    nc.scalar.activation(
                        out=asum[:, :, 0:w], in_=t_ps[:, :, 0:w],
                        func=mybir.ActivationFunctionType.Abs,
                        accum_out=acc[:, col : col + 1],
                    )
                col += 1

        # t_site losses (tiny, issued after the loop so they don't block the
        # DVE FIFO on the tsite DMA): d = gt - pred; t_center = |d0|+|d1|;
        # t_depth = |d2|
        nc.vector.tensor_sub(d_sb, ts_sb[:, 0:3], ts_sb[:, 3:6])
        nc.vector.tensor_reduce(
            out=out_sb[:, 1:2], in_=d_sb[:, 0:2], axis=mybir.AxisListType.X,
            op=mybir.AluOpType.add, apply_absolute_value=True,
        )
        nc.vector.tensor_reduce(
            out=out_sb[:, 2:3], in_=d_sb[:, 2:3], axis=mybir.AxisListType.X,
            op=mybir.AluOpType.add, apply_absolute_value=True,
        )
        nc.vector.tensor_reduce(
            out=out_sb[:, 0:1], in_=acc[:, 0:col], axis=mybir.AxisListType.X,
            op=mybir.AluOpType.add,
        )
        nc.sync.dma_start(out=out, in_=out_sb)

    nc.compile()
    _CACHE["nc"] = nc
    return nc


def _prepare_in_maps(obj_id, gt_cam_R_m2c, pred_cam_R_m2c, gt_cam_t_m2c_site,
                     pred_cam_t_m2c_site, obj_points, obj_diameters):
    obj_id = np.asarray(obj_id).astype(np.int64)
    dR = (np.asarray(pred_cam_R_m2c, np.float32)
          - np.asarray(gt_cam_R_m2c, np.float32))          # [N, 3, 3] (i, j)
    pts = np.asarray(obj_points, np.float32)               # [8, P, 3]

    import ml_dtypes

    # A[(o,j), (i,n)] = [obj_id[n]==o] * dR[n, i, j]
    afull = np.zeros((NUM_OBJECTS, 3, 3, N_SAMPLES), np.float32)  # [o, j, i, n]
    afull[obj_id, :, :, np.arange(N_SAMPLES)] = dR.transpose(0, 2, 1)  # [n, j, i]
    a24 = afull.reshape(NUM_OBJECTS * 3, 3 * N_SAMPLES)    # rows (o,j), cols i*128+n

    # B rows (o,j), cols p
    b24 = pts.transpose(0, 2, 1).reshape(NUM_OBJECTS * 3, NUM_POINTS)

    ts_host = np.concatenate(
        [np.asarray(gt_cam_t_m2c_site, np.float32),
         np.asarray(pred_cam_t_m2c_site, np.float32)], axis=1)  # [128, 6]

    quad_cols = [DVE_COLS, DVE_COLS, ACT_COLS, ACT_COLS]
    in_maps = []
    for c in range(N_CORES):
        cols = b24[:, c * PTS_PER_CORE : (c + 1) * PTS_PER_CORE]
        ab = np.zeros((128, AB_COLS), np.float32)
        off = 0
        for g in range(4):
            w = quad_cols[g]
            ab[32 * g : 32 * g + 24, 0:A_COLS] = a24
            ab[32 * g : 32 * g + 24, A_COLS : A_COLS + w] = cols[:, off : off + w]
            off += w
        in_maps.append({
            "abmat": np.ascontiguousarray(ab).astype(ml_dtypes.bfloat16),
            "tsite": ts_host,
        })
    return in_maps, obj_id, np.asarray(obj_diameters, np.float32)


def _postprocess(results, obj_id, obj_diameters):
    pm_sum = np.zeros(N_SAMPLES, np.float64)
    for c in range(N_CORES):
        pm_sum += results[c]["out"][:, 0].astype(np.float64)
    pm = (pm_sum / NUM_POINTS / obj_diameters[obj_id].astype(np.float64)).astype(
        np.float32)
    t_center = results[0]["out"][:, 1].astype(np.float32)
    t_depth = results[0]["out"][:, 2].astype(np.float32)
    return pm, t_center, t_depth


def run(inputs, trace=False):
    """Run on the 8 NeuronCores. Returns ((pm, t_center, t_depth), BassKernelResults)."""
    from concourse.bass_utils import run_bass_kernel_spmd

    nc = _build_module()
    in_maps, obj_id, diam = _prepare_in_maps(**inputs)
    res = run_bass_kernel_spmd(nc, in_maps, list(range(N_CORES)), trace=trace)
    return _postprocess(res.results, obj_id, diam), res


def run_sim(inputs):
    """CoreSim path (numerics check without hardware)."""
    from concourse.bass_interp import CoreSim

    nc = _build_module()
    in_maps, obj_id, diam = _prepare_in_maps(**inputs)
    results = []
    for c in range(N_CORES):
        sim = CoreSim(nc)
        for name, val in in_maps[c].items():
            sim.tensor(name)[:] = val
        sim.simulate(check_with_hw=False)
        results.append({"out": np.array(sim.tensor("out"))})
    return _postprocess(results, obj_id, diam)


def kernel(**inputs):
    (pm, t_center, t_depth), _ = run(inputs, trace=False)
    return pm, t_center, t_depth
